# revision 73
# baseline (speedup 1.0000x reference)
"""Trainium2 Bass kernel for nn_H_DYNA_42348377538865 (scatter_memory GRU + memory attention).

Self-contained: shards node dim N=512 across 8 NeuronCores (64 nodes/core),
runs a fully-unrolled 24-step recurrence per core, gathers on host.

Layout: feature-on-partitions, (node, batch) on free dim (col = n_local*32 + b,
NB=2048 cols/core, 4 column-chunks of 512). Key decompositions:
  - rolling q-cache: q(h_t) computed once/step; 12 slots in 3x[128,2048] tiles;
    slot j pairs with memory slice s=(j-t)%12 -> 12 precomputed rotation stacks;
    all-zero cache groups skipped in early encode steps
  - single act table: sigmoid(x) = 0.5*(1+tanh(x/2)); exp/tanh/copy all live
    in the `exp_and_others` table -> zero act-table reloads
  - z/r gates in ONE [65,128] matmul + ONE tanh; GRU update via
    scalar_tensor_tensor: 2rh=(1+v)h (0.5 folded into Wc), w=hc-h, s=(1+u)w,
    h+=0.5s; w lives at base partition 64 so stt inputs share a base; in
    decode the (1+v) expands into the candidate matmuls instead
    (cwfx = 0.5Wc[1:]+Wo Wc[0:1] on h, cwf on v*h) so the elementwise op
    is a 2x-mode tensor_mul rather than a mode-less stt
  - softmax without max-subtraction; fu/su matmuls split and PACKED over
    chunk-pairs to 128 psum rows so reciprocal+normalize run at full DVE
    width; nsw (and its base-64 stationary copy) consume the packed halves
    via tile_position moving offsets
  - q bias bq folded into the exp bias (sum_s bq.mem, constant per mem row)
  - decode: Wo folded into gate weights (no autoregressive y->x loopback);
    y accumulates in a persistent PSUM bank via per-step one-hot [64,12]
    stationaries, copied out once at the last step
  - t=0 is constant (h=0): attention context precomputed on host (ctx0),
    injected by a one-hot node-indicator matmul; x enters via a base-0
    rank-1 row so startup never waits on the x-row DMA chain
  - hand-interleaved per-step emission: per-chunk tails (w/s/hupd/qp/qcopy)
    retire as soon as each chunk's hc lands, letting consecutive steps
    pipeline per chunk; engine assignment tuned (w on DVE for chunks 0,1 /
    Pool for 2,3; qcopy chunk 0 on DVE, rest on ACT)
  - hypernet nsw = node_emb @ weight_pool precomputed on host (param repack);
    small weights/biases packed into two DMAs, critical-first DMA order
(The dormant e2_*/H2 "packed decode" path measured slower - 2-way pair
parallelism exposes the dependency chain - and is left uncalled.)
"""
import numpy as np
import sys

for _p in ("/opt/trn_rl_repo",):
    if _p not in sys.path:
        sys.path.append(_p)

import concourse.bass as bass
import concourse.bacc as bacc
import concourse.mybir as mybir
import concourse.tile as tile
from concourse import bass_utils

B, T, HORIZON, N = 32, 12, 12, 512
IN, OUT, H, P = 1, 1, 64, 32
S, ML, MG, DE = 12, 64, 32, 10
NCORES = 8
NL = N // NCORES        # 64
NB = NL * B             # 2048
NSTEP = T + HORIZON     # 24
CH = 4                  # column chunks
CW = NB // CH           # 512

F32 = mybir.dt.float32
BF16 = mybir.dt.bfloat16
AF = mybir.ActivationFunctionType
ALU = mybir.AluOpType


def build_nc():
    nc = bacc.Bacc("TRN2", target_bir_lowering=False, debug=False)
    d = {}
    d["xsrc"] = nc.dram_tensor("xsrc", [T, NB], BF16, kind="ExternalInput")
    d["memstack"] = nc.dram_tensor("memstack", [128, S * 3 * 96], BF16, kind="ExternalInput")
    d["nsw"] = nc.dram_tensor("nsw", [128, NL * 64], BF16, kind="ExternalInput")
    # all small weights packed into one bf16 tensor, all biases into one f32
    d["wpack"] = nc.dram_tensor("wpack", [128, 1776], BF16, kind="ExternalInput")
    d["bpack"] = nc.dram_tensor("bpack", [128, 8], F32, kind="ExternalInput")
    ys_d = nc.dram_tensor("ys", [HORIZON, NB], BF16, kind="ExternalOutput")

    with tile.TileContext(nc) as tc:
        with (
            tc.tile_pool(name="consts", bufs=1) as cp,
            tc.tile_pool(name="sp", bufs=4) as sp,
            tc.tile_pool(name="ps", bufs=2, space="PSUM") as pp,
            tc.tile_pool(name="py", bufs=1, space="PSUM") as py,
        ):
            wpk = cp.tile([128, 1776], BF16)
            nc.sync.dma_start(wpk[:], d["wpack"].ap())
            bpk = cp.tile([128, 8], F32)
            nc.sync.dma_start(bpk[:], d["bpack"].ap())
            xs = cp.tile([T, NB], BF16)
            nc.sync.dma_start(xs[:], d["xsrc"].ap())
            # nsw duplicated on rows 64:128: the packed-fn matmuls for odd
            # chunks read moving at base partition 64, and HW requires the
            # stationary to start at the same partition
            nsw = cp.tile([128, NL * 64], BF16)
            nc.sync.dma_start(nsw[:], d["nsw"].ap())
            msk = cp.tile([128, S * 3 * 96], BF16)
            nc.sync.dma_start(msk[:], d["memstack"].ap())
            fms = wpk[0:96, 0:128]
            zrw = wpk[0:65, 128:256]
            zrwf = wpk[0:64, 256:384]
            cws = wpk[0:65, 384:448]
            cwf = wpk[0:64, 448:512]
            cwx = wpk[0:64, 512:576]
            qw = wpk[0:64, 576:608]
            owd = wpk[0:64, 608:752]
            bqlog = bpk[0:96, 0:1]
            bzr2 = bpk[0:128, 1:2]
            bzrf2 = bpk[0:128, 2:3]
            bce = bpk[0:64, 3:4]
            bcd = bpk[0:64, 4:5]
            brfp = bpk[0:128, 5:6]
            bzfp = bpk[0:128, 6:7]
            bcdp = bpk[0:128, 7:8]
            zrx0 = wpk[0:1, 752:880]
            cwfx = wpk[0:64, 1712:1776]
            cwx0 = wpk[0:1, 880:944]
            # decode H2 path: stationaries duplicated at base partition 64
            rwf0 = wpk[0:64, 256:320]
            zwf0 = wpk[0:64, 320:384]
            rwf64 = wpk[64:128, 256:320]
            zwf64 = wpk[64:128, 320:384]
            cwf64 = wpk[64:128, 448:512]
            cwx64 = wpk[64:128, 512:576]
            qw64 = wpk[64:128, 576:608]
            owd64 = wpk[64:128, 608:752]
            # t=0 attention context is a constant (h=0): ctx0 per node,
            # injected via one-hot node-indicator matmuls
            ctx0 = [wpk[0:16, 944 + 64 * c : 1008 + 64 * c] for c in range(CH)]
            ind16 = wpk[0:16, 1200:1712]

            qb = []
            for g in range(3):
                q = cp.tile([128, NB], BF16, name=f"qb{g}")
                (nc.vector if g == 0 else nc.gpsimd).memset(q[:], 0.0)
                qb.append(q)
            hx = cp.tile([65, NB], BF16)
            nc.vector.memset(hx[:], 0.0)
            rhx = cp.tile([65, NB], BF16)
            nc.vector.memset(rhx[:], 0.0)
            # decode h storage packed to 128 partitions: rows 0:64 = chunks
            # 0,1 (cols 0:1024), rows 64:128 = chunks 2,3 (cols 1024:2048)
            H2 = cp.tile([128, NB // 2], BF16)
            rhx2 = cp.tile([128, NB // 2], BF16)
            # y staging: row 32c + d holds decode step d of chunk c
            # (32-aligned chunk bases for the ACT copy)
            ysb = cp.tile([128, CW], BF16)

            # persistent PSUM: decode y accumulator (chunk c rows 32c:32c+12;
            # each decode step adds Wo^T h into row 32c+d and +0 elsewhere)
            ypt = py.tile([128, CW], F32)

            csl = [slice(c * CW, (c + 1) * CW) for c in range(CH)]

            for t in range(NSTEP):
                r = t % S
                j = t % S
                g_w, row_w = j // 4, (j % 4) * 32
                enc = t <= T  # t==12 still uses x-row (x = source[:, -1])
                # groups whose cache slots are still all-zero are skipped
                # (slot j is first written at the end of step j)
                glist = [g for g in range(3) if t > 4 * g] or [0]
                gprev = (((t - 1) % S) // 4) if t > 0 else 0
                glist = [g for g in glist if g != gprev] + (
                    [gprev] if gprev in glist else [])

                zrp, uvl = [], []
                lgp, exl = [None] * CH, [None] * CH
                ful, sul, rtl, fnl = [], [], [], []
                accl, hcl, wl, sl = [], [], [], []

                def e_zr(c):
                    zp = pp.tile([128, CW], F32, tag="zr", bufs=2, name="zp")
                    if t == 0:
                        # h == 0: only the x row contributes
                        nc.tensor.matmul(zp[:], zrx0, xs[0:1, csl[c]],
                                         start=True, stop=True)
                    elif enc:
                        nc.tensor.matmul(zp[:], zrw, hx[:, csl[c]],
                                         start=True, stop=True)
                    else:
                        nc.tensor.matmul(zp[:], zrwf, hx[0:64, csl[c]],
                                         start=True, stop=True)
                    zrp.append(zp)

                def e_lg(c):
                    if t == 0:
                        return
                    lg = pp.tile([128, CW], F32, tag="lg", bufs=2, name="lg")
                    for gi, g in enumerate(glist):
                        off = (r * 3 + g) * 96
                        nc.tensor.matmul(
                            lg[0:96, :], msk[:, off : off + 96], qb[g][:, csl[c]],
                            start=(gi == 0), stop=(gi == len(glist) - 1),
                        )
                    lgp[c] = lg

                def e_uv(c):
                    uv = sp.tile([128, CW], BF16, tag="uv", bufs=6, name="uv")
                    nc.scalar.activation(uv[:], zrp[c][:], AF.Tanh,
                                         bias=(bzr2 if enc else bzrf2),
                                         scale=0.5)
                    uvl.append(uv)

                def e_ex(c):
                    if t == 0:
                        return
                    ex = sp.tile([96, CW], BF16, tag="ex", bufs=6, name="ex")
                    nc.scalar.activation(ex[:], lgp[c][0:96, :], AF.Exp,
                                         bias=bqlog)
                    exl[c] = ex

                def e_rh(c):
                    if t == 0:
                        return
                    if enc:
                        nc.vector.scalar_tensor_tensor(
                            rhx[0:64, csl[c]], uvl[c][0:64, :], 1.0,
                            hx[0:64, csl[c]], ALU.add, ALU.mult)
                    else:
                        nc.vector.tensor_mul(rhx[0:64, csl[c]], uvl[c][0:64, :],
                                             hx[0:64, csl[c]])

                def e_fusu(pr):
                    if t == 0:
                        return
                    ca, cb = 2 * pr, 2 * pr + 1
                    su = pp.tile([128, CW], F32, tag="fa", bufs=3, name="su")
                    nc.tensor.matmul(su[0:64, :], fms[:, 64:128], exl[ca][:],
                                     start=True, stop=True)
                    nc.tensor.matmul(su[64:128, :], fms[:, 64:128], exl[cb][:],
                                     start=True, stop=True, tile_position=(0, 64))
                    fu = pp.tile([128, CW], F32, tag="fa", bufs=3, name="fu")
                    nc.tensor.matmul(fu[0:64, :], fms[:, 0:64], exl[ca][:],
                                     start=True, stop=True)
                    nc.tensor.matmul(fu[64:128, :], fms[:, 0:64], exl[cb][:],
                                     start=True, stop=True, tile_position=(0, 64))
                    ful.append(fu)
                    sul.append(su)

                def e_rtfn(pr):
                    if t == 0:
                        return
                    rt = sp.tile([128, CW], F32, tag="rt", bufs=3, name="rt")
                    nc.vector.reciprocal_approx_fast(rt[:], sul[pr][:])
                    rtl.append(rt)
                    fn = sp.tile([128, CW], BF16, tag="fn", bufs=3, name="fn")
                    nc.vector.tensor_mul(fn[:], ful[pr][:], rtl[pr][:])
                    fnl.append(fn)

                e2_rtfn = e_rtfn

                def e_cw(c):
                    acc = pp.tile([64, CW], F32, tag="fa", bufs=3, name="acc")
                    if t == 0:
                        nc.tensor.matmul(acc[:], cwx0, xs[0:1, csl[c]],
                                         start=True, stop=False,
                                         skip_group_check=True)
                    elif enc:
                        nc.tensor.matmul(acc[:], cws, rhx[:, csl[c]],
                                         start=True, stop=False,
                                         skip_group_check=True)
                    else:
                        # (0.5Wc[1:]+Wo Wc0)^T h + 0.5Wc[1:]^T (v*h)
                        nc.tensor.matmul(acc[:], cwfx, hx[0:64, csl[c]],
                                         start=True, stop=False,
                                         skip_group_check=True)
                        nc.tensor.matmul(acc[:], cwf, rhx[0:64, csl[c]],
                                         start=False, stop=False,
                                         skip_group_check=True)
                    accl.append(acc)

                def e_nsw(c):
                    if t == 0:
                        nc.tensor.matmul(
                            accl[c][:], ctx0[c], ind16,
                            start=False, stop=True, skip_group_check=True)
                        return
                    pr, half = c // 2, (c % 2) * 64
                    for k in range(16):
                        n = c * 16 + k
                        nc.tensor.matmul(
                            accl[c][:, k * 32 : (k + 1) * 32],
                            nsw[half : half + 64, n * 64 : (n + 1) * 64],
                            fnl[pr][half : half + 64, k * 32 : (k + 1) * 32],
                            start=False, stop=(k == 15), skip_group_check=True,
                            tile_position=(half, 0),
                        )

                def e_hc(c):
                    hc = sp.tile([64, CW], BF16, tag="hc", bufs=5, name="hc")
                    nc.scalar.activation(hc[:], accl[c][:], AF.Tanh,
                                         bias=(bce if enc else bcd))
                    hcl.append(hc)

                def e_w(c):
                    w = sp.tile([128, CW], BF16, tag="w", bufs=5, name="w")
                    eng = nc.vector if c < 2 else nc.gpsimd
                    eng.tensor_sub(w[64:128, :], hcl[c][:], hx[0:64, csl[c]])
                    wl.append(w)

                def e_s(c):
                    s2 = sp.tile([64, CW], BF16, tag="s2", bufs=5, name="s2")
                    nc.vector.scalar_tensor_tensor(
                        s2[:], uvl[c][64:128, :], 1.0, wl[c][64:128, :],
                        ALU.add, ALU.mult)
                    sl.append(s2)

                def e_hupd(c):
                    nc.vector.scalar_tensor_tensor(
                        hx[0:64, csl[c]], sl[c][:], 0.5, hx[0:64, csl[c]],
                        ALU.mult, ALU.add)

                qpb = (pp.tile([128, CW], F32, tag="lg", bufs=2, name="qpb")
                       if t < NSTEP - 1 else None)

                def e_qp(c):
                    if t < NSTEP - 1:
                        nc.tensor.matmul(
                            qpb[32 * c : 32 * (c + 1), :], qw,
                            hx[0:64, csl[c]], start=True, stop=True,
                            tile_position=(0, 32 * c),
                        )
                    if t >= T:
                        dstep = t - T
                        nc.tensor.matmul(
                            ypt[32 * c : 32 * c + HORIZON, :],
                            owd[:, HORIZON * dstep : HORIZON * (dstep + 1)],
                            hx[0:64, csl[c]],
                            start=(dstep == 0), stop=(dstep == HORIZON - 1),
                            skip_group_check=True,
                            tile_position=(0, 32 * c),
                        )

                def e_qcopy(c):
                    if t == NSTEP - 1:
                        nc.scalar.activation(
                            ysb[32 * c : 32 * c + HORIZON, :],
                            ypt[32 * c : 32 * c + HORIZON, :], AF.Copy)
                        nc.sync.dma_start(
                            ys_d.ap()[0:HORIZON, c * CW : (c + 1) * CW],
                            ysb[32 * c : 32 * c + HORIZON, :])
                    if t < NSTEP - 1:
                        dst = qb[g_w][row_w : row_w + 32, csl[c]]
                        src_ = qpb[32 * c : 32 * (c + 1), :]
                        if c == 0:
                            nc.vector.tensor_copy(dst, src_)
                        else:
                            nc.scalar.activation(dst, src_, AF.Copy)

                # ===== decode (t>T): h packed in H2, pair P = (chunk P, chunk P+2) =====
                psl = [slice(0, CW), slice(CW, 2 * CW)]
                vpl, upl, rpp, zpp, accp, hcp, wp2, sp2 = [], [], [], [], [], [], [], []

                def e2_gate(p, z):
                    gp = pp.tile([128, CW], F32, tag="zr", bufs=2, name="gp")
                    w0, w64 = (zwf0, zwf64) if z else (rwf0, rwf64)
                    nc.tensor.matmul(gp[0:64, :], w0, H2[0:64, psl[p]],
                                     start=True, stop=True)
                    nc.tensor.matmul(gp[64:128, :], w64, H2[64:128, psl[p]],
                                     start=True, stop=True, tile_position=(64, 64))
                    (zpp if z else rpp).append(gp)

                def e2_v(p):
                    v = sp.tile([128, CW], BF16, tag="uv", bufs=6, name="v2")
                    nc.scalar.activation(v[:], rpp[p][:], AF.Tanh, bias=brfp,
                                         scale=0.5)
                    vpl.append(v)

                def e2_u(p):
                    u = sp.tile([128, CW], BF16, tag="uv", bufs=6, name="u2")
                    nc.scalar.activation(u[:], zpp[p][:], AF.Tanh, bias=bzfp,
                                         scale=0.5)
                    upl.append(u)

                def e2_rh(p):
                    nc.vector.scalar_tensor_tensor(
                        rhx2[:, psl[p]], vpl[p][:], 1.0, H2[:, psl[p]],
                        ALU.add, ALU.mult)

                def e2_fusu(p):
                    ca, cb = p, p + 2
                    su = pp.tile([128, CW], F32, tag="fa", bufs=3, name="su")
                    nc.tensor.matmul(su[0:64, :], fms[:, 64:128], exl[ca][:],
                                     start=True, stop=True)
                    nc.tensor.matmul(su[64:128, :], fms[:, 64:128], exl[cb][:],
                                     start=True, stop=True, tile_position=(0, 64))
                    fu = pp.tile([128, CW], F32, tag="fa", bufs=3, name="fu")
                    nc.tensor.matmul(fu[0:64, :], fms[:, 0:64], exl[ca][:],
                                     start=True, stop=True)
                    nc.tensor.matmul(fu[64:128, :], fms[:, 0:64], exl[cb][:],
                                     start=True, stop=True, tile_position=(0, 64))
                    ful.append(fu)
                    sul.append(su)

                def e2_cwnsw(p):
                    acc = pp.tile([128, CW], F32, tag="fa", bufs=3, name="acc2")
                    nc.tensor.matmul(acc[0:64, :], cwf, rhx2[0:64, psl[p]],
                                     start=True, stop=False, skip_group_check=True)
                    nc.tensor.matmul(acc[64:128, :], cwf64, rhx2[64:128, psl[p]],
                                     start=True, stop=False, skip_group_check=True,
                                     tile_position=(64, 64))
                    nc.tensor.matmul(acc[0:64, :], cwx, H2[0:64, psl[p]],
                                     start=False, stop=False, skip_group_check=True)
                    nc.tensor.matmul(acc[64:128, :], cwx64, H2[64:128, psl[p]],
                                     start=False, stop=False, skip_group_check=True,
                                     tile_position=(64, 64))
                    for half, c in ((0, p), (64, p + 2)):
                        for k in range(16):
                            n = c * 16 + k
                            nc.tensor.matmul(
                                acc[half : half + 64, k * 32 : (k + 1) * 32],
                                nsw[half : half + 64, n * 64 : (n + 1) * 64],
                                fnl[p][half : half + 64, k * 32 : (k + 1) * 32],
                                start=False, stop=(half == 64 and k == 15),
                                skip_group_check=True,
                                tile_position=(half, half),
                            )
                    accp.append(acc)

                def e2_hc(p):
                    hc = sp.tile([128, CW], BF16, tag="hc", bufs=5, name="hc2")
                    nc.scalar.activation(hc[:], accp[p][:], AF.Tanh, bias=bcdp)
                    hcp.append(hc)

                def e2_w(p):
                    w = sp.tile([128, CW], BF16, tag="w", bufs=5, name="w2")
                    eng = nc.vector if p == 0 else nc.gpsimd
                    eng.tensor_sub(w[:], hcp[p][:], H2[:, psl[p]])
                    wp2.append(w)

                def e2_s(p):
                    s2 = sp.tile([128, CW], BF16, tag="s2", bufs=5, name="s22")
                    nc.vector.scalar_tensor_tensor(
                        s2[:], upl[p][:], 1.0, wp2[p][:], ALU.add, ALU.mult)
                    sp2.append(s2)

                def e2_hupd(p):
                    nc.vector.scalar_tensor_tensor(
                        H2[:, psl[p]], sp2[p][:], 0.5, H2[:, psl[p]],
                        ALU.mult, ALU.add)

                def e2_qpy(c):
                    p, half = c % 2, (c // 2) * 64
                    mv = H2[half : half + 64, psl[p]]
                    qws = qw if half == 0 else qw64
                    if t < NSTEP - 1:
                        nc.tensor.matmul(
                            qpb[32 * c : 32 * (c + 1), :], qws, mv,
                            start=True, stop=True,
                            tile_position=(half, 32 * c),
                        )
                    dstep = t - T
                    ow0 = owd if half == 0 else owd64
                    nc.tensor.matmul(
                        ypt[32 * c : 32 * c + HORIZON, :],
                        ow0[:, HORIZON * dstep : HORIZON * (dstep + 1)], mv,
                        start=(dstep == 0), stop=(dstep == HORIZON - 1),
                        skip_group_check=True,
                        tile_position=(half, 32 * c),
                    )

                if True:  # 4-chunk path for all steps (H2 pair path below
                    # measured slower: 2-way parallelism exposes the chain)
                    # hand-interleaved emission: front-load pair-0 attention
                    # path, finish each chunk as soon as its hc lands
                    e_lg(0); e_lg(1)
                    e_zr(0); e_zr(1); e_zr(2); e_zr(3)
                    e_ex(0); e_ex(1)
                    e_fusu(0)
                    e_uv(0)
                    e_rtfn(0)
                    e_rh(0)
                    e_uv(1)
                    e_rh(1)
                    e_lg(2); e_lg(3)
                    e_uv(2); e_uv(3)
                    e_ex(2); e_ex(3)
                    e_rh(2); e_rh(3)
                    e_cw(0); e_cw(1)
                    e_nsw(0)
                    e_fusu(1)
                    e_hc(0)
                    e_rtfn(1)
                    e_nsw(1)
                    e_hc(1)
                    e_w(0); e_s(0); e_hupd(0); e_qp(0); e_qcopy(0)
                    e_cw(2); e_cw(3)
                    e_nsw(2)
                    e_hc(2)
                    e_w(1); e_s(1); e_hupd(1); e_qp(1); e_qcopy(1)
                    e_nsw(3)
                    e_hc(3)
                    e_w(2); e_s(2); e_hupd(2); e_qp(2); e_qcopy(2)
                    e_w(3); e_s(3); e_hupd(3); e_qp(3); e_qcopy(3)

                else:
                    e2_gate(0, False); e2_gate(1, False)
                    e2_v(0); e2_v(1)
                    e_lg(0); e_lg(2)
                    e2_rh(0); e2_rh(1)
                    e_ex(0); e_ex(2)
                    e2_gate(0, True); e2_gate(1, True)
                    e2_fusu(0)
                    e_lg(1); e_lg(3)
                    e2_rtfn(0)
                    e_ex(1); e_ex(3)
                    e2_u(0); e2_u(1)
                    e2_fusu(1)
                    e2_cwnsw(0)
                    e2_rtfn(1)
                    e2_hc(0)
                    e2_cwnsw(1)
                    e2_w(0); e2_s(0); e2_hupd(0)
                    e2_hc(1)
                    e2_qpy(0); e2_qpy(2); e_qcopy(0); e_qcopy(2)
                    e2_w(1); e2_s(1); e2_hupd(1)
                    e2_qpy(1); e2_qpy(3); e_qcopy(1); e_qcopy(3)

                # --- DMA: encode x prefetch ---
                if t < T - 1:
                    nc.sync.dma_start(hx[64:65, :], xs[t + 1 : t + 2, :])
                    nc.sync.dma_start(rhx[64:65, :], xs[t + 1 : t + 2, :])


    nc.compile()
    return nc


def precompute(inp):
    lm = np.asarray(inp["local_mem"], np.float64)
    gm = np.asarray(inp["global_mem"], np.float64)
    Wq = np.asarray(inp["Wq"], np.float64)
    bq = np.asarray(inp["bq"], np.float64)
    node_emb = np.asarray(inp["node_emb"], np.float64)
    wp = np.asarray(inp["weight_pool"], np.float64)
    Wz = np.asarray(inp["Wz"], np.float64)
    bz = np.asarray(inp["bz"], np.float64)
    Wr = np.asarray(inp["Wr"], np.float64)
    br = np.asarray(inp["br"], np.float64)
    Wc = np.asarray(inp["Wc"], np.float64)
    bc = np.asarray(inp["bc"], np.float64)
    Wo = np.asarray(inp["Wo"], np.float64)
    bo = np.asarray(inp["bo"], np.float64)

    c = {}
    c["nsw_full"] = np.einsum("nd,dfh->nfh", node_emb, wp)
    memsl = np.concatenate([lm.transpose(2, 0, 1), gm.transpose(2, 0, 1)], axis=1)  # [P,96,S]
    ms = np.zeros((128, S, 3, 96))
    for rr in range(S):
        for g in range(3):
            for i in range(4):
                s = (4 * g + i - rr) % S
                ms[32 * i : 32 * (i + 1), rr, g, :] = memsl[:, :, s]
    c["memstack"] = ms.reshape(128, S * 3 * 96)
    lmean, gmean = lm.mean(axis=1), gm.mean(axis=1)
    fms = np.zeros((96, 128))
    fms[:ML, :P] = lmean
    fms[ML:, P : 2 * P] = gmean
    fms[:ML, 64 : 64 + P] = 1.0
    fms[ML:, 64 + P : 128] = 1.0
    c["fms"] = fms
    # r-gate block first (cols 0:64) so v sits at base partition 0 next to h
    zrw = np.zeros((H + 1, 128))
    zrw[:H, :H] = Wr[1:]
    zrw[H, :H] = Wr[0]
    zrw[:H, H:] = Wz[1:]
    zrw[H, H:] = Wz[0]
    c["zrw"] = zrw
    Wzf = Wz[1:] + Wo @ Wz[0:1]
    Wrf = Wr[1:] + Wo @ Wr[0:1]
    c["zrwf"] = np.concatenate([Wrf, Wzf], axis=1)
    cws = np.zeros((H + 1, H))
    cws[:H] = 0.5 * Wc[1:]
    cws[H] = Wc[0]
    c["cws"] = cws
    c["cwf"] = 0.5 * Wc[1:]
    c["cwx"] = Wo @ Wc[0:1]
    c["qw"] = Wq.copy()
    owd = np.zeros((H, HORIZON * HORIZON))
    for dd in range(HORIZON):
        owd[:, HORIZON * dd + dd] = Wo[:, 0]
    c["owd"] = owd
    c["bqlog"] = np.concatenate([lm.sum(axis=1) @ bq, gm.sum(axis=1) @ bq]).reshape(96, 1)
    c["bzr2"] = (0.5 * np.concatenate([br, bz])).reshape(128, 1)
    c["bzrf2"] = (0.5 * np.concatenate([br + bo[0] * Wr[0], bz + bo[0] * Wz[0]])).reshape(128, 1)
    c["bce"] = bc.reshape(64, 1)
    c["bcd"] = (bc + bo[0] * Wc[0]).reshape(64, 1)
    # t=0 constant attention context (h=0 -> logits = bqlog)
    bl = c["bqlog"][:, 0]
    al = np.exp(bl[:ML] - bl[:ML].max()); al /= al.sum()
    ag = np.exp(bl[ML:] - bl[ML:].max()); ag /= ag.sum()
    fused0 = np.concatenate([al @ lmean, ag @ gmean])
    c["ctx0_full"] = np.einsum("f,nfh->nh", fused0, c["nsw_full"])
    c["bo"] = float(bo[0])
    return c


def _bf16(a):
    import ml_dtypes
    return np.ascontiguousarray(a).astype(ml_dtypes.bfloat16)


def _f32(a):
    return np.ascontiguousarray(a).astype(np.float32)


def make_in_maps(inp):
    c = precompute(inp)
    src = np.asarray(inp["source"], np.float32)
    wpack = np.zeros((128, 1776))
    wpack[0:96, 0:128] = c["fms"]
    wpack[0:65, 128:256] = c["zrw"]
    wpack[0:64, 256:384] = c["zrwf"]
    wpack[0:65, 384:448] = c["cws"]
    wpack[0:64, 448:512] = c["cwf"]
    wpack[0:64, 512:576] = c["cwx"]
    wpack[0:64, 576:608] = c["qw"]
    wpack[0:64, 608:752] = c["owd"]
    wpack[0:64, 1712:1776] = c["cwf"] + c["cwx"]
    wpack[0:1, 752:816] = c["zrw"][64:65, 0:64]
    wpack[0:1, 816:880] = c["zrw"][64:65, 64:128]
    wpack[0:1, 880:944] = c["cws"][64:65, :]
    # base-64 duplicates for the packed decode path
    wpack[64:128, 256:384] = c["zrwf"]
    wpack[64:128, 448:512] = c["cwf"]
    wpack[64:128, 512:576] = c["cwx"]
    wpack[64:128, 576:608] = c["qw"]
    wpack[64:128, 608:752] = c["owd"]
    for i in range(16):
        wpack[i, 1200 + i * 32 : 1200 + (i + 1) * 32] = 1.0
    bpack = np.zeros((128, 8))
    bpack[0:96, 0] = c["bqlog"][:, 0]
    bpack[0:128, 1] = c["bzr2"][:, 0]
    bpack[0:128, 2] = c["bzrf2"][:, 0]
    bpack[0:64, 3] = c["bce"][:, 0]
    bpack[0:64, 4] = c["bcd"][:, 0]
    bpack[0:128, 5] = np.tile(c["bzrf2"][0:64, 0], 2)   # r-gate bias, both halves
    bpack[0:128, 6] = np.tile(c["bzrf2"][64:128, 0], 2) # z-gate bias, both halves
    bpack[0:128, 7] = np.tile(c["bcd"][:, 0], 2)
    shared = {"memstack": _bf16(c["memstack"]), "bpack": _f32(bpack)}
    in_maps = []
    for core in range(NCORES):
        nodes = slice(core * NL, (core + 1) * NL)
        xsc = _bf16(src[:, :, nodes, 0].transpose(1, 2, 0).reshape(T, NB))
        nswc1 = c["nsw_full"][nodes].transpose(1, 0, 2).reshape(64, NL * 64)
        nswc = _bf16(np.concatenate([nswc1, nswc1], axis=0))
        wpc = wpack.copy()
        ctx0c = c["ctx0_full"][nodes]  # [NL, 64]
        for ch in range(4):
            wpc[0:16, 944 + 64 * ch : 1008 + 64 * ch] = ctx0c[ch * 16 : (ch + 1) * 16]
        in_maps.append(dict(shared, xsrc=xsc, nsw=nswc, wpack=_bf16(wpc)))
    return in_maps


_BO_CACHE = {}


def assemble(results, bo=0.0):
    out = np.zeros((B, HORIZON, N, OUT), np.float32)
    for core in range(NCORES):
        nodes = slice(core * NL, (core + 1) * NL)
        ys = np.asarray(results[core]["ys"], np.float32) + bo  # [HORIZON, NB]
        out[:, :, nodes, 0] = ys.reshape(HORIZON, NL, B).transpose(2, 0, 1)
    return out


_NC_CACHE = {}


def kernel(**inputs):
    if "nc" not in _NC_CACHE:
        _NC_CACHE["nc"] = build_nc()
    nc = _NC_CACHE["nc"]
    in_maps = make_in_maps(inputs)
    bo = float(np.asarray(inputs["bo"], np.float64)[0])
    res = bass_utils.run_bass_kernel_spmd(nc, in_maps, core_ids=list(range(NCORES)))
    return assemble(res.results, bo)
